# revision 3
# baseline (speedup 1.0000x reference)
"""Trainium2 Bass kernel for nn_CrossModalGatedBottleneckAttention (v2).

Contract: kernel(**inputs) takes the FULL unsharded inputs (as produced by
the problem's setup_inputs) and returns the full [16, 768, 512] output.

Strategy: data parallelism over batch B=16 across 8 NeuronCores (2 batches
per core). v2 redesign vs v1:
  - all activations/weights in fp16 (tolerance 2e-2; fp16 keeps ~1e-3):
    halves SBUF so both local batches' working sets fit concurrently,
    enables DVE 2x/4x modes, 1 cyc/row PE transposes, halves DMA bytes
  - weights DMA'd once (not per batch); x_2 loaded once per batch
  - per-batch-parity tile tags so the Tile scheduler overlaps batch n's
    ACT-heavy MHA softmax phase with batch n+1's PE-heavy projections
  - MHA QK logits for a (head, kv-tile) go into one 2-bank PSUM supertile
    so each softmax exp is a single [128,768] ACT op (halves ACT op count)
  - PE transposes write 8 x [128,128] fp16 blocks into one PSUM bank,
    drained by a single wide DVE copy
  - proj bias via tensor_tensor add (frees PE bias matmuls)
"""
import sys as _sys
for _p in ("/opt/trn_rl_repo",):
    if _p not in _sys.path:
        _sys.path.insert(0, _p)

import numpy as np
import concourse.bass as bass
import concourse.mybir as mybir
import concourse.tile as tile
from concourse.bass_utils import run_bass_kernel_spmd
from concourse.masks import make_identity

# ---------------------------------------------------------------------------
# Workaround for walrus sync-wait encoding limits: several instruction
# encodings in this neuronxcc build reject more than one sem-wait per
# instruction ("Too many sync wait commands"). After Tile scheduling, move
# all but one wait of each instruction onto same-engine NoOps inserted just
# before it. An engine blocks on each wait in order, so semantics are
# preserved.
_wsplit_ctr = [0]


def _split_waits(nc, max_waits=1):
    n_split = 0
    for f in nc.m.functions:
        for blk in f.blocks:
            insts = blk.instructions
            new_list = []
            changed = False
            for inst in insts:
                si = inst.sync_info
                if si is not None and si.on_wait and len(si.on_wait) > max_waits:
                    waits = list(si.on_wait)
                    extra, keep = waits[:-max_waits], waits[-max_waits:]
                    for w in extra:
                        _wsplit_ctr[0] += 1
                        nop = mybir.InstNoOp(
                            name=f"I-wsplit-{_wsplit_ctr[0]}", ins=[], outs=[])
                        nop.engine = inst.engine
                        nop.sync_info = mybir.SyncInfo(on_wait=[w], on_update=[])
                        new_list.append(nop)
                        n_split += 1
                    inst.sync_info = mybir.SyncInfo(
                        on_wait=keep, on_update=list(si.on_update or []))
                    changed = True
                new_list.append(inst)
            if changed:
                insts.clear()
                insts.extend(new_list)
    return n_split


# ---------------------------------------------------------------------------
F32 = mybir.dt.float32
F16 = mybir.dt.float16
AF = mybir.ActivationFunctionType
ALU = mybir.AluOpType

F = 512
N = 768
NB = 256
NHEADS = 8
HD = 64
KT = F // 128           # 4 k-tiles over feature dim
NT = N // 128           # 6 tiles over tokens
NBT = NB // 128         # 2 tiles over bottleneck tokens
SCALE = float(F) ** -0.5
MHA_SCALE = float(HD) ** -0.5
B_LOC = 2               # batches per core


def mm_acc(nc, psum_ap, pairs):
    n = len(pairs)
    for i, (l, r) in enumerate(pairs):
        nc.tensor.matmul(psum_ap, l, r, start=(i == 0), stop=(i == n - 1))


def build(nc: bass.Bass, repeat: int = 1):
    dram = {}

    def din(name, shape, dt=F16):
        dram[name] = nc.declare_dram_parameter(name, list(shape), dt,
                                               isOutput=False)
        return dram[name]

    for name, shape in [
            ("x1t", [B_LOC, F, N]), ("x2t", [B_LOC, F, N]), ("zbt", [F, NB]),
            ("wkv_i", [F, 2 * F]), ("wq_j", [F, F]), ("wqkv_b", [F, 3 * F]),
            ("w_f", [2 * F, F]), ("w_m", [F, F]),
            ("w_qkv", [F, 3 * F]), ("w_proj", [F, F])]:
        din(name, shape)
    din("b_f", [F], F32)
    din("b_m", [F], F32)
    din("bproj_rep", [128, F], F32)
    out = nc.declare_dram_parameter("out", [B_LOC, N, F], F32, isOutput=True)

    with tile.TileContext(nc) as tc:
        if repeat == 1:
            _body(nc, tc, dram, out)
        else:
            with tc.For_i(0, repeat, 1):
                _body(nc, tc, dram, out)
    return nc


def _wview(ap):
    # [Fin, Fout] dram -> [128, Fin//128, Fout] partition-tiled view
    return ap[:, :].rearrange("(k p) o -> p k o", p=128)


def _xview(ap):
    # [F, N] dram -> [128, KT, N]
    return ap.rearrange("(k p) n -> p k n", p=128)


def _body(nc, tc, dram, out):
    import contextlib
    with contextlib.ExitStack() as ctx:
        consts = ctx.enter_context(tc.tile_pool(name="consts", bufs=1))
        wts = ctx.enter_context(tc.tile_pool(name="wts", bufs=1))
        acts = ctx.enter_context(tc.tile_pool(name="acts", bufs=1))
        smalls = ctx.enter_context(tc.tile_pool(name="smalls", bufs=1))
        pp_st = ctx.enter_context(tc.tile_pool(name="pp_st", bufs=1, space="PSUM"))
        pp_mm = ctx.enter_context(tc.tile_pool(name="pp_mm", bufs=1, space="PSUM"))
        pp_h = ctx.enter_context(tc.tile_pool(name="pp_h", bufs=1, space="PSUM"))
        _emit(nc, tc, dram, out, consts, wts, acts, smalls, pp_st, pp_mm, pp_h)


def _transpose_group(nc, smalls, pp_mm, ident, blocks, dst_ap_fn, tag, p):
    """Transpose a list of [128,128] f16 SBUF blocks via PE into shared
    PSUM f16 supertiles (up to 8 blocks per bank), draining each supertile
    with one wide DVE copy. blocks: list of src APs; dst_ap_fn(i0, n) must
    return a contiguous dst AP covering blocks i0..i0+n-1 (n*128 cols)."""
    i = 0
    nblk = len(blocks)
    while i < nblk:
        n = min(8, nblk - i)
        pt = pp_mm.tile([128, 1024], F16, tag="mm", bufs=3, name=f"tr_{tag}")
        for j in range(n):
            nc.tensor.transpose(pt[:, j * 128:(j + 1) * 128], blocks[i + j],
                                ident[:, :])
        nc.vector.tensor_copy(dst_ap_fn(i, n), pt[:, :n * 128])
        i += n


def _small_attention(nc, smalls, pp_mm, pp_st, ident, lhs, rhs_fm, A_T, p):
    """A = 0.5 * softmax(lhs.T @ rhs_fm * SCALE, axis=-1) written to A_T
    transposed [NB, N] f16. Generator: yields at chunk boundaries."""
    E = smalls.tile([128, NT, NB], F16, tag=f"esm{p}")
    for nt in range(NT):
        ps = pp_mm.tile([128, 512], F32, tag="mm", bufs=3)
        mm_acc(nc, ps[:, :NB],
               [(lhs[:, k, nt * 128:(nt + 1) * 128], rhs_fm[:, k, :])
                for k in range(KT)])
        srs = smalls.tile([128, 2], F32, tag=f"srs{p}", bufs=4)
        nc.scalar.activation(out=E[:, nt, :], in_=ps[:, :NB], func=AF.Exp,
                             scale=SCALE, accum_out=srs[:, 0:1])
        nc.vector.reciprocal(srs[:, 1:2], srs[:, 0:1])
        nc.vector.tensor_scalar(out=E[:, nt, :], in0=E[:, nt, :],
                                scalar1=srs[:, 1:2], scalar2=0.5,
                                op0=ALU.mult, op1=ALU.mult)
        if nt % 3 == 2:
            yield
    # transposes: per mt, 6 [128,128] blocks -> one PSUM bank -> wide copy
    for mt in range(NBT):
        pt = pp_mm.tile([128, 1024], F16, tag="mm", bufs=3, name="tr_sm")
        for j in range(NT):
            nc.tensor.transpose(pt[:, j * 128:(j + 1) * 128],
                                E[:, j, mt * 128:(mt + 1) * 128], ident[:, :])
        nc.vector.tensor_copy(A_T[:, mt, :], pt[:, :NT * 128])
        yield


def _emit(nc, tc, dram, out, consts, wts, acts, smalls, pp_st, pp_mm, pp_h):
    # ---- constants -------------------------------------------------
    ident = consts.tile([128, 128], F16)
    make_identity(nc, ident)
    bf_c = consts.tile([128, KT], F32)
    nc.sync.dma_start(out=bf_c, in_=dram["b_f"][:].rearrange("(k p) -> p k", p=128))
    bm_c = consts.tile([128, KT], F32)
    nc.sync.dma_start(out=bm_c, in_=dram["b_m"][:].rearrange("(k p) -> p k", p=128))
    bproj_s = consts.tile([128, F], F32)
    nc.sync.dma_start(out=bproj_s, in_=dram["bproj_rep"][:, :])
    zrow = consts.tile([1, 390], F16)
    nc.vector.memset(zrow[:, :], 0.0)
    ones_h = consts.tile([1, 128], F16)
    nc.vector.memset(ones_h[:, :], 1.0)

    # ---- prologue: bottleneck projections (batch independent) ------
    # DMA issue order = queue order: first the tensors feeding the first PE
    # work (zbt, wqkv_b), then both batches' inputs so batch 1's loads do
    # not queue behind batch 0's output stores, then weights by first use.
    zbt_s = smalls.tile([128, KT, NB], F16, tag="zbt")
    nc.sync.dma_start(out=zbt_s, in_=_xview(dram["zbt"][:, :]))
    wqkvb_s = wts.tile([128, KT, 3 * F], F16, tag="w1536")
    nc.sync.dma_start(out=wqkvb_s, in_=_wview(dram["wqkv_b"]))
    xs = []
    for b in range(B_LOC):
        z_it = acts.tile([128, KT, N], F16, tag=f"z_it{b % 2}", name="z_it")
        nc.sync.dma_start(out=z_it, in_=_xview(dram["x1t"][b]))
        z_jt = acts.tile([128, KT, N], F16, tag=f"z_jt{b % 2}", name="z_jt")
        nc.sync.dma_start(out=z_jt, in_=_xview(dram["x2t"][b]))
        xs.append((z_it, z_jt))
    wkvi_s = wts.tile([128, KT, 2 * F], F16, tag="wkvi")
    nc.sync.dma_start(out=wkvi_s, in_=_wview(dram["wkv_i"]))
    wqj_s = wts.tile([128, KT, F], F16, tag="wqj")
    nc.sync.dma_start(out=wqj_s, in_=_wview(dram["wq_j"]))
    wf_s = wts.tile([128, 2 * KT, F], F16, tag="wf")
    nc.sync.dma_start(out=wf_s, in_=_wview(dram["w_f"]))
    wm_s = wts.tile([128, KT, F], F16, tag="wm")
    nc.sync.dma_start(out=wm_s, in_=_wview(dram["w_m"]))
    wproj_s = wts.tile([128, KT, F], F16, tag="wproj")
    nc.sync.dma_start(out=wproj_s, in_=_wview(dram["w_proj"]))

    q_bT = consts.tile([128, KT, NB], F16)
    k_bT = consts.tile([128, KT, NB], F16)
    for dst, co in ((q_bT, 0), (k_bT, F)):
        for mt in range(KT):
            ps = pp_mm.tile([128, 512], F32, tag="mm", bufs=3)
            mm_acc(nc, ps[:, :NB],
                   [(wqkvb_s[:, k, co + mt * 128: co + (mt + 1) * 128],
                     zbt_s[:, k, :]) for k in range(KT)])
            nc.vector.tensor_scalar_mul(dst[:, mt, :], ps[:, :NB], 0.2)
    v_b = consts.tile([128, NBT, F], F16)
    for mt in range(NBT):
        ps = pp_mm.tile([128, 512], F32, tag="mm", bufs=3)
        mm_acc(nc, ps[:, :],
               [(zbt_s[:, k, mt * 128:(mt + 1) * 128],
                 wqkvb_s[:, k, 2 * F:3 * F]) for k in range(KT)])
        nc.vector.tensor_scalar_mul(v_b[:, mt, :], ps[:, :], 0.2)

    # w_qkv reuses the wqkv_b slot once the prologue is done with it
    wqkv_s = wts.tile([128, KT, 3 * F], F16, tag="w1536")
    nc.sync.dma_start(out=wqkv_s, in_=_wview(dram["w_qkv"]))

    # MHA V layout: per token-tile, 8 heads x (64 V cols + 1 ones col)
    V_plus = smalls.tile([128, NT, NHEADS * (HD + 1)], F16, tag="vplus")
    Vp_h = V_plus.rearrange("p n (h c) -> p n h c", c=HD + 1)
    nc.vector.memset(Vp_h[:, :, :, HD], 1.0)

    # ---- per-batch pipeline ---------------------------------------
    # Engines execute their queues in issue order, so cross-batch overlap
    # must exist in the issued stream: batch b's ACT-bound MHA head chunks
    # are interleaved with batch b+1's PE-bound P1-P5 chunks below.
    def _front(b):
        p = b % 2
        # P1: projections (k_i, q_j feature-major; v_i token-major)
        z_it, z_jt = xs[b]

        k_iT = acts.tile([128, KT, N], F16, tag=f"kA{p}")
        q_jT = acts.tile([128, KT, N], F16, tag=f"qF{p}")
        for dst, wsrc, wco, xsrc in ((k_iT, wkvi_s, 0, z_it),
                                     (q_jT, wqj_s, 0, z_jt)):
            for mt in range(KT):
                for nh in range(2):
                    ps = pp_mm.tile([128, 512], F32, tag="mm", bufs=3)
                    mm_acc(nc, ps[:, :384],
                           [(wsrc[:, k, wco + mt * 128:wco + (mt + 1) * 128],
                             xsrc[:, k, nh * 384:(nh + 1) * 384])
                            for k in range(KT)])
                    nc.vector.tensor_copy(dst[:, mt, nh * 384:(nh + 1) * 384],
                                          ps[:, :384])
                yield
        v_i = acts.tile([128, NT, F], F16, tag=f"v_i{p}")
        for nt in range(NT):
            ps = pp_mm.tile([128, 512], F32, tag="mm", bufs=3)
            mm_acc(nc, ps[:, :],
                   [(z_it[:, k, nt * 128:(nt + 1) * 128],
                     wkvi_s[:, k, F:2 * F]) for k in range(KT)])
            nc.scalar.copy(v_i[:, nt, :], ps[:, :])
            if nt % 2 == 1:
                yield

        # P2: a_ib attention (softmax over kv = N tokens, free axis)
        ctm = smalls.tile([128, NBT, F], F16, tag=f"ctm{p}")
        cfm = smalls.tile([128, KT, NB], F16, tag=f"cfm{p}")
        A_ibT = smalls.tile([128, NT, NB], F16, tag=f"aib{p}")
        Eib = smalls.tile([128, NBT, N], F16, tag=f"esm{p}")
        for mt in range(NBT):
            st = pp_st.tile([128, 1024], F32, tag="st", bufs=2)
            for nh in range(2):
                mm_acc(nc, st[:, nh * 512:nh * 512 + 384],
                       [(q_bT[:, k, mt * 128:(mt + 1) * 128],
                         k_iT[:, k, nh * 384:(nh + 1) * 384])
                        for k in range(KT)])
            srs = smalls.tile([128, 2], F32, tag=f"srs{p}", bufs=4)
            E = Eib[:, mt]
            nc.scalar.activation(
                out=E[:, :].rearrange("q (c x) -> q c x", c=2),
                in_=st[:, :].rearrange("q (c x) -> q c x", c=2)[:, :, 0:384],
                func=AF.Exp, scale=SCALE, accum_out=srs[:, 0:1])
            nc.vector.reciprocal(srs[:, 1:2], srs[:, 0:1])
            nc.vector.tensor_scalar_mul(E[:, :], E[:, :], srs[:, 1:2])
            # transpose E rows into A_ibT columns: 6 blocks
            pt = pp_mm.tile([128, 1024], F16, tag="mm", bufs=3, name="tr_ib")
            for j in range(NT):
                nc.tensor.transpose(pt[:, j * 128:(j + 1) * 128],
                                    E[:, j * 128:(j + 1) * 128], ident[:, :])
            nc.vector.tensor_copy(
                A_ibT[:, :, mt * 128:(mt + 1) * 128],
                pt[:, :NT * 128].rearrange("q (n x) -> q n x", x=128))
            yield
        for mt in range(NBT):
            ps = pp_mm.tile([128, 512], F32, tag="mm", bufs=3)
            mm_acc(nc, ps[:, :],
                   [(A_ibT[:, nt, mt * 128:(mt + 1) * 128], v_i[:, nt, :])
                    for nt in range(NT)])
            nc.vector.tensor_copy(ctm[:, mt, :], ps[:, :])
        # ctx feature-major via transposes of ctx_tm (8 blocks, one bank)
        pt = pp_mm.tile([128, 1024], F16, tag="mm", bufs=3, name="tr_cf")
        for ft in range(KT):
            for mt in range(NBT):
                nc.tensor.transpose(
                    pt[:, (ft * NBT + mt) * 128:(ft * NBT + mt + 1) * 128],
                    ctm[:, mt, ft * 128:(ft + 1) * 128], ident[:, :])
        nc.vector.tensor_copy(cfm[:, :, :].rearrange("q a x -> q (a x)"),
                              pt[:, :])
        yield

        # P3: the two [N, NB] attentions (softmax over NB free axis)
        A_upT = smalls.tile([128, NBT, N], F16, tag=f"aup{p}")
        yield from _small_attention(nc, smalls, pp_mm, pp_st, ident,
                                    lhs=z_it, rhs_fm=cfm, A_T=A_upT, p=p)
        A_bjT = smalls.tile([128, NBT, N], F16, tag=f"abj{p}")
        yield from _small_attention(nc, smalls, pp_mm, pp_st, ident,
                                    lhs=q_jT, rhs_fm=k_bT, A_T=A_bjT, p=p)

        # P4: a_ij fm (0.5 folded into A_upT/A_bjT); reuses k_iT's slot
        aijT = acts.tile([128, KT, N], F16, tag=f"kA{p}")
        for ft in range(KT):
            for nh in range(2):
                ps = pp_mm.tile([128, 512], F32, tag="mm", bufs=3)
                pairs = [(ctm[:, mt, ft * 128:(ft + 1) * 128],
                          A_upT[:, mt, nh * 384:(nh + 1) * 384])
                         for mt in range(NBT)]
                pairs += [(v_b[:, mt, ft * 128:(ft + 1) * 128],
                           A_bjT[:, mt, nh * 384:(nh + 1) * 384])
                          for mt in range(NBT)]
                mm_acc(nc, ps[:, :384], pairs)
                nc.vector.tensor_copy(aijT[:, ft, nh * 384:(nh + 1) * 384],
                                      ps[:, :384])
            yield

        # P5: gated fusion; f_T reuses q_jT's slot
        f_T = acts.tile([128, KT, N], F16, tag=f"qF{p}")
        for ft in range(KT):
            st = pp_st.tile([128, 1024], F32, tag="st", bufs=2)
            for nh in range(2):
                pairs = [(wf_s[:, k, ft * 128:(ft + 1) * 128],
                          aijT[:, k, nh * 384:(nh + 1) * 384])
                         for k in range(KT)]
                pairs += [(wf_s[:, KT + k, ft * 128:(ft + 1) * 128],
                           z_jt[:, k, nh * 384:(nh + 1) * 384])
                          for k in range(KT)]
                mm_acc(nc, st[:, nh * 512:nh * 512 + 384], pairs)
            nc.scalar.activation(
                out=f_T[:, ft, :].rearrange("q (c x) -> q c x", c=2),
                in_=st[:, :].rearrange("q (c x) -> q c x", c=2)[:, :, 0:384],
                func=AF.Sigmoid, bias=bf_c[:, ft:ft + 1], scale=1.0)
            if ft % 2 == 1:
                yield
        h_T = acts.tile([128, KT, N], F16, tag=f"h{p}")
        for ft in range(KT):
            utmp = smalls.tile([128, N], F16, tag=f"utmp{p}", bufs=2)
            for nh in range(2):
                ps = pp_mm.tile([128, 512], F32, tag="mm", bufs=3)
                mm_acc(nc, ps[:, :384],
                       [(wm_s[:, k, ft * 128:(ft + 1) * 128],
                         aijT[:, k, nh * 384:(nh + 1) * 384])
                        for k in range(KT)])
                nc.vector.scalar_tensor_tensor(
                    out=utmp[:, nh * 384:(nh + 1) * 384], in0=ps[:, :384],
                    scalar=bm_c[:, ft:ft + 1],
                    in1=f_T[:, ft, nh * 384:(nh + 1) * 384],
                    op0=ALU.add, op1=ALU.mult)
            nc.vector.tensor_tensor(out=utmp[:, :], in0=utmp[:, :],
                                    in1=z_it[:, ft, :], op=ALU.add)
            nc.vector.tensor_scalar_max(h_T[:, ft, :], utmp[:, :], 0.0)
            if ft % 2 == 1:
                yield
        state[b] = h_T

    state = {}

    def _qkv(b):
        h_T = state[b]
        # P6a: Q/K/V projections (shared tags; serialize with prior batch)
        Q_T = acts.tile([128, KT, N], F16, tag="QT")
        K_T = acts.tile([128, KT, N], F16, tag="KT")
        for dst, co in ((Q_T, 0), (K_T, F)):
            for mt in range(KT):
                for nh in range(2):
                    ps = pp_mm.tile([128, 512], F32, tag="mm", bufs=3)
                    mm_acc(nc, ps[:, :384],
                           [(wqkv_s[:, k, co + mt * 128:co + (mt + 1) * 128],
                             h_T[:, k, nh * 384:(nh + 1) * 384])
                            for k in range(KT)])
                    nc.vector.tensor_copy(dst[:, mt, nh * 384:(nh + 1) * 384],
                                          ps[:, :384])
        for nt in range(NT):
            ps = pp_mm.tile([128, 512], F32, tag="mm", bufs=3)
            mm_acc(nc, ps[:, :],
                   [(h_T[:, k, nt * 128:(nt + 1) * 128],
                     wqkv_s[:, k, 2 * F:3 * F]) for k in range(KT)])
            nc.vector.tensor_copy(Vp_h[:, nt, :, 0:HD],
                                  ps[:, :].rearrange("p (h c) -> p h c", c=HD))
        state[b] = (Q_T, K_T)

    def _mha(b):
        Q_T, K_T = state[b]
        H_tm = acts.tile([128, NT, F], F16, tag="Htm")
        for h in range(NHEADS):
            po = 64 * (h % 2)
            kt = h // 2
            Qh = Q_T[po:po + 64, kt, :]
            Kh = K_T[po:po + 64, kt, :]
            hp = pp_h.tile([128, 390], F32, tag="hp", bufs=1, name="hp")
            # zero the bank once; every AV matmul accumulates (interleaved
            # qt groups can't each own a start=True: it marks the whole
            # 2KB zero region pending)
            nc.tensor.matmul(hp[:, :], ones_h[0:1, :], zrow[0:1, :],
                             start=True, stop=False, skip_group_check=True)
            for kv in range(NT):
                st = pp_st.tile([128, 1024], F32, tag="st", bufs=2)
                nc.tensor.matmul(st[:, 0:512],
                                 Kh[:, kv * 128:(kv + 1) * 128],
                                 Qh[:, 0:512],
                                 start=True, stop=True)
                nc.tensor.matmul(st[:, 512:768],
                                 Kh[:, kv * 128:(kv + 1) * 128],
                                 Qh[:, 512:768],
                                 start=True, stop=True)
                et = smalls.tile([128, N], F16, tag="et", bufs=4)
                nc.scalar.activation(out=et[:, :], in_=st[:, 0:768],
                                     func=AF.Exp, scale=MHA_SCALE)
                for qt in range(NT):
                    nc.tensor.matmul(
                        hp[:, qt * 65:(qt + 1) * 65],
                        et[:, qt * 128:(qt + 1) * 128],
                        Vp_h[:, kv, h, :],
                        start=False, stop=(kv == NT - 1),
                        skip_group_check=True)
            hp_q = hp[:, :].rearrange("p (q c) -> p q c", c=65)
            rt = smalls.tile([128, NT], F32, tag="rt", bufs=2)
            nc.vector.reciprocal(rt[:, :], hp_q[:, :, 64])
            for qt in range(NT):
                nc.vector.tensor_scalar_mul(
                    H_tm[:, qt, h * 64:(h + 1) * 64],
                    hp_q[:, qt, 0:64], rt[:, qt:qt + 1])
            # H_T transposes for feature-tile ft become ready after heads
            # 2ft and 2ft+1
            if h % 2 == 1:
                ft = h // 2
                if h == 1:
                    H_T = acts.tile([128, KT, N], F16, tag="HT")
                pt = pp_mm.tile([128, 1024], F16, tag="mm", bufs=3,
                                name="tr_h")
                for nt in range(NT):
                    nc.tensor.transpose(pt[:, nt * 128:(nt + 1) * 128],
                                        H_tm[:, nt, ft * 128:(ft + 1) * 128],
                                        ident[:, :])
                nc.vector.tensor_copy(H_T[:, ft, :], pt[:, :NT * 128])
            yield

        for nt in range(NT):
            ps = pp_mm.tile([128, 512], F32, tag="mm", bufs=3)
            mm_acc(nc, ps[:, :],
                   [(H_T[:, k, nt * 128:(nt + 1) * 128], wproj_s[:, k, :])
                    for k in range(KT)])
            osb = smalls.tile([128, F], F32, tag="osb", bufs=2)
            nc.vector.tensor_tensor(out=osb[:, :], in0=ps[:, :],
                                    in1=bproj_s[:, :], op=ALU.add)
            nc.sync.dma_start(out=out[b, nt * 128:(nt + 1) * 128, :],
                              in_=osb[:, :])

    # drive: interleave batch 0 and batch 1 fronts 2:1 (batch 0 stays
    # ahead), then batch 0's MHA heads interleaved with the rest of batch
    # 1's front, then batch 1's tail. Shared-tag (QT/KT/Vp/Htm) users are
    # issued strictly after the prior batch's last reader.
    _SENT = object()
    f0 = _front(0)
    f1 = _front(1)
    done_f = False
    while True:
        if next(f0, _SENT) is _SENT:
            break
        if next(f0, _SENT) is _SENT:
            break
        if not done_f and next(f1, _SENT) is _SENT:
            done_f = True
    _qkv(0)
    m0 = _mha(0)
    done_m = False
    while not (done_m and done_f):
        if not done_m and next(m0, _SENT) is _SENT:
            done_m = True
        for _ in range(3):
            if not done_f and next(f1, _SENT) is _SENT:
                done_f = True
    _qkv(1)
    for _ in _mha(1):
        pass


# ---------------------------------------------------------------------------
# Host-side wrapper
N_CORES = 8
_nc_cache = {}


def _get_nc(repeat=1):
    if repeat not in _nc_cache:
        nc = bass.Bass("TRN2", num_devices=N_CORES)
        build(nc, repeat=repeat)
        _split_waits(nc)
        _nc_cache[repeat] = nc
    return _nc_cache[repeat]


def _host_prep_shared(inputs):
    f16 = np.float16

    def c(a, dt=f16):
        return np.ascontiguousarray(np.asarray(a, np.float32).astype(dt))

    return {
        "zbt": c(np.asarray(inputs["z_b"]).T),
        "wkv_i": c(np.asarray(inputs["Wqkv_i"])[:, F:]),
        "wq_j": c(np.asarray(inputs["Wqkv_j"])[:, :F]),
        "wqkv_b": c(inputs["Wqkv_b"]),
        "w_f": c(inputs["W_f"]),
        "w_m": c(inputs["W_m"]),
        "w_qkv": c(inputs["W_QKV"]),
        "w_proj": c(inputs["W_proj"]),
        "b_f": np.ascontiguousarray(np.asarray(inputs["b_f"], np.float32)),
        "b_m": np.ascontiguousarray(np.asarray(inputs["b_m"], np.float32)),
        "bproj_rep": np.ascontiguousarray(
            np.tile(np.asarray(inputs["b_proj"], np.float32).reshape(1, F),
                    (128, 1))),
    }


def make_in_maps(inputs):
    x1 = np.asarray(inputs["x_1"], np.float32)
    x2 = np.asarray(inputs["x_2"], np.float32)
    B = x1.shape[0]
    assert B == N_CORES * B_LOC, (B, N_CORES, B_LOC)
    shared = _host_prep_shared(inputs)
    in_maps = []
    for c in range(N_CORES):
        sl = slice(c * B_LOC, (c + 1) * B_LOC)
        m = dict(shared)
        m["x1t"] = np.ascontiguousarray(
            x1[sl].transpose(0, 2, 1).astype(np.float16))
        m["x2t"] = np.ascontiguousarray(
            x2[sl].transpose(0, 2, 1).astype(np.float16))
        in_maps.append(m)
    return in_maps


def kernel(**inputs) -> np.ndarray:
    nc = _get_nc(repeat=1)
    in_maps = make_in_maps(inputs)
    res = run_bass_kernel_spmd(nc, in_maps, list(range(N_CORES)))
    out = np.concatenate([np.asarray(r["out"]) for r in res.results], axis=0)
    return out.astype(np.float32)
